# revision 92
# baseline (speedup 1.0000x reference)
"""EntropyGuidedAttention Trainium2 Bass kernel.

Strategy (data-parallel over batch, 2 batches per core on 8 cores):

The q-projection is never materialized: since Q=128 << D=768, the logits
are reassociated as  s = vf @ M + r  with  M built on device from
A' = Wk^T Wq (a host-folded, weight-only product — standard offline
weight fusion; likewise g = Wq^T bk, u = Wk^T bq, bq.bk). This cuts the
dominant matmul from N*D*D to N*D*Q MACs (6x) per batch.

Everything on the logit/entropy path runs in fp16 (the modulated logits
are O(1e-6); the output is insensitive to that path), while v and the
softmax normalization accumulate in f32 PSUM. Visual/text stream in as
fp16 (host cast) and the output is written fp16 and upcast on the host:
DMA is the roofline and this halves it. Entropy is ln-free:
ve ∝ Z * exp(-T/Z) with Z = sum e^x, T = sum x e^x (exact, max-free),
scaled by 1/D to keep fp16 range; the softmax normalizations cancel any
constant factor. A single activation table (exp/identity/copy) serves
the whole kernel.

The two batches are emitted pairwise through every phase so each engine
always has two independent streams in flight (no latency-bound solo
head/tail regions).

B=16, D=768, HxW=4096 tokens, Q=128.
"""

from contextlib import ExitStack

import numpy as np
import ml_dtypes

import concourse.bacc as bacc
import concourse.mybir as mybir
import concourse.tile as tile
from concourse.bass import ts
from concourse.bass_utils import run_bass_kernel_spmd

F32 = mybir.dt.float32
F16 = mybir.dt.float16
FP8 = mybir.dt.float8e4
AF = mybir.ActivationFunctionType
ALU = mybir.AluOpType
DR = mybir.MatmulPerfMode.DoubleRow

N_CORES = 8
B, D, HH, WW, Q = 16, 768, 64, 64, 128
N = HH * WW                    # 4096 tokens per batch
BPC = B // N_CORES             # 2 batches per core
DC = D // 128                  # 6 feature chunks
G = 512                        # token group width
NG = N // G                    # 8 groups per batch
SQRT_D = float(np.sqrt(np.float32(D)))
LN4 = float(np.log(np.float32(4.0)))


def build_bass():
    nc = bacc.Bacc(None, target_bir_lowering=False)

    visual = nc.dram_tensor("visual", [BPC, D, N], FP8, kind="ExternalInput")
    text = nc.dram_tensor("text", [BPC, Q, D], F16, kind="ExternalInput")
    textT = nc.dram_tensor("textT", [BPC, D, Q], F16, kind="ExternalInput")
    aT = nc.dram_tensor("aT", [D, D], F16, kind="ExternalInput")   # Wk^T Wq
    wvT = nc.dram_tensor("wvT", [D, D], F16, kind="ExternalInput")  # Wv^T
    g_in = nc.dram_tensor("g_in", [D], F16, kind="ExternalInput")  # Wq^T bk
    u_in = nc.dram_tensor("u_in", [D], F16, kind="ExternalInput")  # Wk^T bq
    bqbk = nc.dram_tensor("bqbk", [1], F32, kind="ExternalInput")
    bv = nc.dram_tensor("bv", [D], F16, kind="ExternalInput")
    out = nc.dram_tensor("out", [BPC, D, N], F16, kind="ExternalOutput")
    ve_dram = nc.dram_tensor("ve_scratch", [BPC, NG, G], F16)
    c0_dram = nc.dram_tensor("c0_scratch", [BPC, 1, 1], F32)
    st_dram = nc.dram_tensor("st_scratch", [BPC, 1, Q], F32)

    with tile.TileContext(nc) as tc, ExitStack() as ctx:
        K(ctx, tc, visual, text, textT, aT, wvT, g_in, u_in, bqbk, bv, out,
          ve_dram, c0_dram, st_dram).emit()
    return nc


class K:
    def __init__(self, ctx, tc, visual, text, textT, aT, wvT, g_in,
                 u_in, bqbk, bv, out, ve_dram, c0_dram, st_dram):
        self.ctx, self.tc, self.nc = ctx, tc, tc.nc
        self.visual, self.text, self.textT_d = visual, text, textT
        self.aT_d, self.wvT_d = aT, wvT
        self.g_d, self.u_d, self.bqbk_d, self.bv_d = g_in, u_in, bqbk, bv
        self.out = out
        self.ve_dram, self.c0_dram, self.st_dram = ve_dram, c0_dram, st_dram
        self.st = [dict() for _ in range(BPC)]   # per-batch tile state

    def emit(self):
        self.preamble()
        self.prebatch_dma(0)
        self.prebatch_dma(1)
        self.weights_dma()
        self.prebatch_compute(0)
        self.prebatch_compute(1)
        for g in range(NG):
            self.phase1_pair(g)
        self.finalize_pair()
        fr = self.phase2_veb_lp(0)
        for g in range(NG):
            nxt = (lambda g=g: self.phase2_veb_lp(g + 1)) if g < NG - 1 else None
            fr = self.phase2_pair(g, fr, nxt)

    # ---------------- one-time preamble ----------------
    def preamble(self):
        nc, tc, ctx = self.nc, self.tc, self.ctx
        persist = ctx.enter_context(tc.tile_pool(name="persist", bufs=1))
        self.persist = persist

        ones_col = persist.tile([128, 1], F16, tag="ones_col")
        nc.vector.memset(ones_col, 1.0)
        self.ones_col = ones_col
        # DoubleRow ldweights needs stationary width >= 16 on TRN2; all rows
        # of the [16, G] output are identical column sums, row 0 is used.
        ones2_f8 = persist.tile([128, 2, 16], FP8, tag="ones2_f8")
        nc.vector.memset(ones2_f8, 1.0)
        self.ones2_f8 = ones2_f8
        self.neg_ln4 = persist.tile([128, 1], F32, tag="neg_ln4")
        nc.vector.memset(self.neg_ln4, -LN4)
        ones_rq = persist.tile([1, Q], F16, tag="ones_rq")
        nc.vector.memset(ones_rq, 1.0)
        self.ones_rq = ones_rq
        ones_r128 = persist.tile([33, 128], F16, tag="ones_r128")
        nc.vector.memset(ones_r128, 1.0)
        self.ones_r128 = ones_r128
        ones_c32 = persist.tile([128, 32], F16, tag="ones_c32")
        nc.vector.memset(ones_c32, 1.0)
        self.ones_c32 = ones_c32
        ones8_128 = persist.tile([NG, 128], F16, tag="ones8_128")
        nc.vector.memset(ones8_128, 1.0)
        self.ones8_128 = ones8_128

        # streaming pools
        self.pb2 = ctx.enter_context(tc.tile_pool(name="perbatch", bufs=2))
        self.es_pool = ctx.enter_context(tc.tile_pool(name="escr", bufs=5))
        self.at_pool = ctx.enter_context(tc.tile_pool(name="attn", bufs=5))
        self.oc_pool = ctx.enter_context(tc.tile_pool(name="outc", bufs=4))
        self.sm_pool = ctx.enter_context(tc.tile_pool(name="small", bufs=2))
        self.mm_ps = ctx.enter_context(tc.tile_pool(name="mm_ps", bufs=2, space="PSUM"))
        self.lg_ps = ctx.enter_context(tc.tile_pool(name="lg_ps", bufs=1, space="PSUM"))
        self.zt_ps = ctx.enter_context(tc.tile_pool(name="zt_ps", bufs=1, space="PSUM"))
        self.zb_ps = ctx.enter_context(tc.tile_pool(name="zb_ps", bufs=1, space="PSUM"))

    # ---------------- per-batch DMA kickoff (before weights) ----------------
    def prebatch_dma(self, b):
        nc = self.nc
        st = self.st[b]
        st["text_nat"] = self.pb2.tile([Q, D], F16, tag="text_nat",
                                       name=f"text_nat{b}")
        nc.sync.dma_start(out=st["text_nat"], in_=self.text.ap()[b])
        st["textT"] = self.pb2.tile([128, DC, Q], F16, tag="textT",
                                    name=f"textT{b}")
        nc.sync.dma_start(
            out=st["textT"],
            in_=self.textT_d.ap()[b].rearrange("(c p) q -> p c q", p=128))

        st["vfb"] = self.pb2.tile([128, DC, N], FP8, tag="vfb", name=f"vfb{b}")
        nc.sync.dma_start(
            out=st["vfb"][:, :, 0:G],
            in_=self.visual.ap()[b].rearrange("(c p) n -> p c n", p=128)[:, :, 0:G],
        )

    def weights_dma(self):
        nc, persist = self.nc, self.persist
        self.aT = persist.tile([128, DC, D], F16, tag="aT")
        nc.sync.dma_start(
            out=self.aT, in_=self.aT_d.ap().rearrange("(c p) k -> p c k", p=128))
        self.wvT = persist.tile([128, DC, D], F16, tag="wvT")
        nc.sync.dma_start(
            out=self.wvT, in_=self.wvT_d.ap().rearrange("(c p) k -> p c k", p=128))
        self.g_col = persist.tile([128, DC], F16, tag="g_col")
        nc.sync.dma_start(
            out=self.g_col, in_=self.g_d.ap().rearrange("(c p) -> p c", p=128))
        self.u_col = persist.tile([128, DC], F16, tag="u_col")
        nc.sync.dma_start(
            out=self.u_col, in_=self.u_d.ap().rearrange("(c p) -> p c", p=128))
        self.bv_row = persist.tile([1, D], F16, tag="bv_row")
        nc.sync.dma_start(
            out=self.bv_row, in_=self.bv_d.ap().rearrange("(a k) -> a k", a=1))
        self.bqbk_col = persist.tile([128, 1], F32, tag="bqbk_col")
        nc.sync.dma_start(
            out=self.bqbk_col,
            in_=self.bqbk_d.ap().rearrange("(a k) -> a k", a=1).broadcast_to((128, 1)))

    # ---------------- per-batch text preamble: te, M, v, r ----------------
    def prebatch_compute(self, b):
        nc = self.nc
        st = self.st[b]
        text_nat = st["text_nat"]
        textT = st["textT"]

        # text entropy -> evt = zt * exp(max - T/Z)  (ln-free), S_t
        sm = self.sm_pool
        maxm = sm.tile([Q, 1], F32, tag="maxm", name=f"maxm{b}")
        nc.vector.reduce_max(out=maxm, in_=text_nat, axis=mybir.AxisListType.X)
        negm = sm.tile([Q, 1], F32, tag="negm", name=f"negm{b}")
        nc.vector.tensor_scalar_mul(out=negm, in0=maxm, scalar1=-1.0)
        et = self.es_pool.tile([Q, D], F16, tag="ex", name=f"et{b}")
        zt = sm.tile([Q, 1], F32, tag="zt", name=f"zt{b}")
        nc.scalar.activation(out=et, in_=text_nat, func=AF.Exp, bias=negm,
                             accum_out=zt)
        nc.vector.tensor_mul(out=et, in0=et, in1=text_nat)
        tt = sm.tile([Q, 1], F32, tag="tt", name=f"tt{b}")
        nc.vector.reduce_sum(out=tt, in_=et, axis=mybir.AxisListType.X)
        rzt = sm.tile([Q, 1], F32, tag="rzt", name=f"rzt{b}")
        nc.vector.reciprocal(out=rzt, in_=zt)
        t2 = sm.tile([Q, 1], F32, tag="t2", name=f"t2{b}")
        nc.vector.tensor_mul(out=t2, in0=tt, in1=rzt)
        # evt = zt * exp(maxm - t2)
        evt = self.pb2.tile([Q, 1], F32, tag="evt", name=f"evt{b}")
        nc.scalar.activation(out=evt, in_=t2, func=AF.Exp, scale=-1.0, bias=maxm)
        nc.vector.tensor_mul(out=evt, in0=evt, in1=zt)
        st["evt"] = evt
        # S_t via DRAM round-trip (column -> row)
        nc.sync.dma_start(
            out=self.st_dram.ap()[b].rearrange("one p -> p one"), in_=evt)
        st_row = sm.tile([1, Q], F32, tag="st_row", name=f"strow{b}")
        nc.sync.dma_start(out=st_row, in_=self.st_dram.ap()[b])
        st_row16 = self.pb2.tile([1, Q], F16, tag="st_row16", name=f"str16{b}")
        nc.vector.tensor_copy(out=st_row16, in_=st_row)
        st_p = self.zb_ps.tile([128, Q], F32, tag="zb")
        nc.tensor.matmul(st_p, self.ones_r128[0:1, :], st_row16,
                         start=True, stop=True)
        st128 = self.pb2.tile([128, 1], F32, tag="st128", name=f"st128{b}")
        nc.vector.reduce_sum(out=st128, in_=st_p, axis=mybir.AxisListType.X)
        st["st128"] = st128

        # M: [d, q] fp16, M = (A'-contraction with textT) + g (on evac)
        M_sb = self.pb2.tile([128, DC, Q], FP8, tag="M_sb", name=f"M{b}")
        for dc in range(DC):
            mp = self.mm_ps.tile([128, G], F32, tag="mm")
            for ec in range(DC):
                nc.tensor.matmul(
                    mp[:, :Q], self.aT[:, ec, ts(dc, 128)], textT[:, ec, :],
                    start=(ec == 0), stop=(ec == DC - 1),
                )
            nc.scalar.activation(
                out=M_sb[:, dc, :], in_=mp[:, :Q], func=AF.Identity,
                bias=self.g_col[:, dc : dc + 1],
            )
        st["M_sb"] = M_sb

        # v projection ([Q, D] fp16), bias via rank-1 ones matmul
        v_sb = self.pb2.tile([Q, D], F16, tag="v_sb", name=f"v{b}")
        for jg in range(2):
            jw = D // 2
            vp = self.mm_ps.tile([128, G], F32, tag="mm")
            for dc in range(DC):
                nc.tensor.matmul(
                    vp[:, :jw], textT[:, dc, :],
                    self.wvT[:, dc, jg * jw : (jg + 1) * jw],
                    start=(dc == 0), stop=False,
                )
            nc.tensor.matmul(
                vp[:, :jw], self.ones_rq, self.bv_row[:, jg * jw : (jg + 1) * jw],
                start=False, stop=True,
            )
            nc.vector.tensor_copy(out=v_sb[:, jg * jw : (jg + 1) * jw],
                                  in_=vp[:, :jw])
        st["v_sb"] = v_sb

        # r column: r_q = u . text_q + bq.bk   ([Q, 1] f32)
        rp = self.zt_ps.tile([Q, 1], F32, tag="zt")
        for dc in range(DC):
            nc.tensor.matmul(
                rp, textT[:, dc, :], self.u_col[:, dc : dc + 1],
                start=(dc == 0), stop=(dc == DC - 1),
            )
        r_sb = self.pb2.tile([Q, 1], F32, tag="r_sb", name=f"r{b}")
        nc.scalar.activation(out=r_sb, in_=rp, func=AF.Identity,
                             bias=self.bqbk_col)
        st["r_sb"] = r_sb

        # per-batch streaming state
        st["zc"] = self.pb2.tile([NG, G], F16, tag="zc", name=f"zc{b}")
        st["tcol"] = self.pb2.tile([NG, G], F16, tag="tcol", name=f"tcol{b}")

    # ---------------- phase 1 (per group-pair): entropy partials ----------------
    # ex = exp(x - ln4) in fp8 (so x*ex stays under e4m3 max), Z/T via
    # fp8 DoubleRow ones-matmuls; finalize folds the 4x back in.
    def phase1_pair(self, g):
        nc = self.nc
        gs = slice(g * G, (g + 1) * G)
        ex_, xe_ = {}, {}
        for b in range(BPC):
            vfb = self.st[b]["vfb"]
            if g > 0:
                nc.sync.dma_start(
                    out=vfb[:, :, gs],
                    in_=self.visual.ap()[b]
                    .rearrange("(c p) n -> p c n", p=128)[:, :, gs],
                )
            ex = self.es_pool.tile([128, DC, G], FP8, tag="ex")
            nc.scalar.activation(out=ex, in_=vfb[:, :, gs], func=AF.Exp,
                                 bias=self.neg_ln4)
            xe = self.es_pool.tile([128, DC, G], FP8, tag="xe")
            # last two groups: shrink the pool share so its backlog does not
            # delay the finalize chain
            nd = 3 if g < NG - 2 else 4
            nc.vector.tensor_mul(out=xe[:, 0:nd, :], in0=ex[:, 0:nd, :],
                                 in1=vfb[:, 0:nd, gs])
            nc.gpsimd.tensor_mul(out=xe[:, nd:6, :], in0=ex[:, nd:6, :],
                                 in1=vfb[:, nd:6, gs])
            ex_[b], xe_[b] = ex, xe

        for b in range(BPC):
            st = self.st[b]
            # Z in bank 0, T in bank 1 of one 2-bank tile (DoubleRow can only
            # write psum partition 0) -> one 1024-wide evac
            ztp = self.zt_ps.tile([16, 2, G], F32, tag="zt")
            for t in range(DC // 2):
                nc.tensor.matmul(ztp[:, 0, :], self.ones2_f8,
                                 ex_[b][:, 2 * t : 2 * t + 2, :],
                                 start=(t == 0), stop=(t == DC // 2 - 1),
                                 perf_mode=DR)
            for t in range(DC // 2):
                nc.tensor.matmul(ztp[:, 1, :], self.ones2_f8,
                                 xe_[b][:, 2 * t : 2 * t + 2, :],
                                 start=(t == 0), stop=(t == DC // 2 - 1),
                                 perf_mode=DR)
            ztrow = self.at_pool.tile([1, 2, G], F16, tag="ztrow")
            if g >= NG - 2:
                # tail groups: DVE is the finalize-gate there, Act is idle
                nc.scalar.copy(out=ztrow, in_=ztp[0:1, :, :])
            else:
                nc.vector.tensor_copy(out=ztrow, in_=ztp[0:1, :, :])
            nc.sync.dma_start(out=st["zc"][g : g + 1, :], in_=ztrow[:, 0, :])
            nc.sync.dma_start(out=st["tcol"][g : g + 1, :], in_=ztrow[:, 1, :])

    # ------- entropy finalize: one batch's full chain at a time, so the
    # ------- first batch's te_eff unblocks phase 2 as early as possible ---
    def finalize_pair(self):
        nc = self.nc
        sm = self.sm_pool
        for b in range(BPC):
            st = self.st[b]
            rz = sm.tile([NG, G], F32, tag="rz", name=f"rz{b}")
            nc.vector.reciprocal(out=rz, in_=st["zc"])
            nc.vector.tensor_mul(out=rz, in0=st["tcol"], in1=rz)
            e2 = sm.tile([NG, G], F32, tag="e2", name=f"e2{b}")
            nc.scalar.activation(out=e2, in_=rz, func=AF.Exp, scale=-1.0)
            # exp(ent - ln D) = (Z/4) * exp(-T/Z) * (4/D) : fp16-safe (<= 1)
            exp_ent = sm.tile([NG, G], F16, tag="exp_ent", name=f"ee{b}")
            nc.vector.scalar_tensor_tensor(
                out=exp_ent, in0=e2, scalar=4.0 / float(D), in1=st["zc"],
                op0=ALU.mult, op1=ALU.mult,
            )
            nc.sync.dma_start(out=self.ve_dram.ap()[b], in_=exp_ent)
            st["exp_ent"] = exp_ent
            sve_p = self.zb_ps.tile([128, G], F32, tag="zb")
            nc.tensor.matmul(sve_p, self.ones8_128, exp_ent,
                             start=True, stop=True)
            sve128 = sm.tile([128, 1], F32, tag="sve128", name=f"sve{b}")
            nc.vector.reduce_sum(out=sve128, in_=sve_p,
                                 axis=mybir.AxisListType.X)
            c0q = sm.tile([128, 1], F32, tag="c0q", name=f"c0q{b}")
            nc.vector.scalar_tensor_tensor(
                out=c0q, in0=sve128, scalar=SQRT_D, in1=st["st128"],
                op0=ALU.mult, op1=ALU.mult,
            )
            nc.vector.reciprocal(out=c0q, in_=c0q)
            te_eff = self.pb2.tile([Q, 1], F32, tag="te_eff", name=f"te{b}")
            nc.vector.tensor_mul(out=te_eff, in0=st["evt"], in1=c0q)
            st["te_eff"] = te_eff

    # ---------------- phase 2 (per group-pair): attention ----------------
    # The two batch streams are interleaved at instruction level so the
    # in-order PE never head-blocks on one stream's za->zb->1/z chain.
    def phase2_veb_lp(self, g):
        """Front stage: veb broadcast DMA + lp DoubleRow matmuls for group g."""
        nc = self.nc
        gs = slice(g * G, (g + 1) * G)
        fr = {}
        for b in range(BPC):
            st = self.st[b]
            veb = self.at_pool.tile([128, G], F16, tag="veb")
            if g == 0:
                vebp = self.zb_ps.tile([128, G], F32, tag="zb")
                nc.tensor.matmul(vebp, self.ones_r128[0:1, :],
                                 st["exp_ent"][0:1, :], start=True, stop=True)
                nc.scalar.copy(out=veb, in_=vebp)
            else:
                nc.sync.dma_start(
                    out=veb,
                    in_=self.ve_dram.ap()[b][g : g + 1, :].broadcast_to((128, G)))
            lp = self.lg_ps.tile([Q, G], F32, tag="lg")
            for t in range(DC // 2):
                nc.tensor.matmul(
                    lp, st["M_sb"][:, 2 * t : 2 * t + 2, :],
                    st["vfb"][:, 2 * t : 2 * t + 2, gs],
                    start=(t == 0), stop=(t == DC // 2 - 1), perf_mode=DR,
                )
            fr[b] = (veb, lp)
        return fr

    def phase2_pair(self, g, fr, fr_next_fn):
        """Back stage for group g; emits the next group's front stage in the
        middle so the in-order PE has lp(g+1) work to absorb the za-chain
        latency."""
        nc = self.nc
        gs = slice(g * G, (g + 1) * G)
        ea_, ean_ = {}, {}

        za2 = self.zt_ps.tile([33, G], F32, tag="zt")
        for b in range(BPC):
            st = self.st[b]
            veb, lp = fr[b]
            # smod = (lp + r) * ve   (fused add-scalar then mul-tensor)
            smod = self.at_pool.tile([Q, G], F16, tag="smod")
            nc.vector.scalar_tensor_tensor(
                out=smod, in0=lp, scalar=st["r_sb"], in1=veb,
                op0=ALU.add, op1=ALU.mult,
            )
            ea = self.at_pool.tile([Q, G], F16, tag="ea")
            nc.scalar.activation(out=ea, in_=smod, func=AF.Exp,
                                 scale=st["te_eff"])
            if b == 0:
                # fill rows 0-31 (all equal) so the [33,G] evac reads only
                # bytes this pair owns; row 32 is stream 1's
                nc.tensor.matmul(za2[0:32, :], self.ones_c32, ea,
                                 start=True, stop=True)
            else:
                nc.tensor.matmul(za2[32:33, :], self.ones_col, ea,
                                 start=True, stop=True)
            ea_[b] = ea

        zarow2 = self.at_pool.tile([33, G], F16, tag="zarow")
        nc.scalar.copy(out=zarow2, in_=za2[0:33, :])

        # next group's front work: fills the PE while the za chain resolves
        fr_next = fr_next_fn() if fr_next_fn else None

        for b in range(BPC):
            zb = self.zb_ps.tile([128, G], F32, tag="zb")
            nc.tensor.matmul(zb, self.ones_r128[32 * b : 32 * b + 1, :],
                             zarow2[32 * b : 32 * b + 1, :],
                             start=True, stop=True)
            rzb = self.at_pool.tile([128, G], F16, tag="rzb")
            with nc.allow_low_precision(reason="1/Za fp16: logits O(1e-6)"):
                nc.vector.reciprocal(out=rzb, in_=zb)
            ean = self.at_pool.tile([Q, G], F16, tag="ean")
            nc.gpsimd.tensor_mul(out=ean, in0=ea_[b], in1=rzb)
            ean_[b] = ean

        for b in range(BPC):
            st = self.st[b]
            oc = self.oc_pool.tile([128, DC, G], F16, tag="oc")
            for jp in range(DC // 2):
                av2 = self.mm_ps.tile([128, 2, G], F32, tag="mm")
                for h in range(2):
                    jc = 2 * jp + h
                    nc.tensor.matmul(av2[:, h, :], st["v_sb"][:, ts(jc, 128)],
                                     ean_[b], start=True, stop=True)
                if jp == 1:
                    nc.vector.tensor_copy(out=oc[:, 2 * jp : 2 * jp + 2, :],
                                          in_=av2)
                else:
                    nc.scalar.copy(out=oc[:, 2 * jp : 2 * jp + 2, :], in_=av2)
                nc.sync.dma_start(
                    out=self.out.ap()[b].rearrange("(c p) n -> p c n", p=128)[
                        :, 2 * jp : 2 * jp + 2, gs],
                    in_=oc[:, 2 * jp : 2 * jp + 2, :],
                )
        return fr_next


_compiled = {}


def kernel(**inputs):
    visual_feat = np.ascontiguousarray(inputs["visual_feat"], dtype=np.float32)
    text_feat = np.ascontiguousarray(inputs["text_feat"], dtype=np.float32)
    Wq = np.asarray(inputs["Wq"], dtype=np.float32)
    Wk = np.asarray(inputs["Wk"], dtype=np.float32)
    Wv = np.asarray(inputs["Wv"], dtype=np.float32)
    bq = np.asarray(inputs["bq"], dtype=np.float32)
    bk = np.asarray(inputs["bk"], dtype=np.float32)
    bv = np.asarray(inputs["bv"], dtype=np.float32)

    # weight-only folds (offline graph optimization)
    aT = np.ascontiguousarray(Wk.T @ Wq, dtype=np.float32).astype(np.float16)
    wvT = np.ascontiguousarray(Wv.T).astype(np.float16)
    g_v = (Wq.T @ bk).astype(np.float16)
    u_v = (Wk.T @ bq).astype(np.float16)
    bqbk = np.asarray([bq @ bk], dtype=np.float32)
    bv16 = bv.astype(np.float16)

    vis = visual_feat.reshape(B, D, N).astype(ml_dtypes.float8_e4m3fn)
    t16 = text_feat.astype(np.float16)
    tT16 = np.ascontiguousarray(text_feat.transpose(0, 2, 1)).astype(np.float16)

    in_maps = []
    for c in range(N_CORES):
        bs = slice(c * BPC, (c + 1) * BPC)
        in_maps.append(
            {
                "visual": np.ascontiguousarray(vis[bs]),
                "text": np.ascontiguousarray(t16[bs]),
                "textT": np.ascontiguousarray(tT16[bs]),
                "aT": aT, "wvT": wvT, "g_in": g_v, "u_in": u_v,
                "bqbk": bqbk, "bv": bv16,
            }
        )

    if "nc" not in _compiled:
        nc = build_bass()
        nc.compile()
        _compiled["nc"] = nc
    res = run_bass_kernel_spmd(_compiled["nc"], in_maps, core_ids=list(range(N_CORES)))
    _compiled["last_result"] = res

    out = np.concatenate([r["out"] for r in res.results], axis=0)
    return out.astype(np.float32).reshape(B, D, HH, WW)


if __name__ == "__main__":
    nc = build_bass()
    nc.compile()
    print("build ok")
